# revision 17
# baseline (speedup 1.0000x reference)
"""Trainium2 Bass kernel for thresholded multi-head attention (v2).

Computes, for x:[b,n,dim] with b=4, n=2048, dim=512, heads=8, dh=64:
    qkv = x @ Wqkv + bqkv ; split q,k,v per head
    dots = q k^T / sqrt(dh) ; attn = softmax(dots)
    attn = where(attn > 0.01, attn, 0) ; out = attn @ v
    return out @ Wout + bout
Sharding over 8 NeuronCores: core c handles batch c//2, head group c%2
(4 heads); host sums the two partial output projections per batch.

v2 design (vs v1): S is computed with QUERIES on partitions so that
  - Z (softmax denominator) falls out of the exp pass via the Act
    engine's accum_out (free-dim reduction) — the whole two-limb Z
    machinery (PE ones-matmuls + fp16 cast + gpsimd residual) is gone;
  - the attn>0.01 compare and the 1/Z normalization fuse into ONE
    custom DVE op with per-partition scalars (no broadcast tiles);
  - P^T (needed by the PV matmul, which contracts over keys) comes from
    the DMA XBAR transpose (idle engine) instead of PE transposes.
S uses the exact fp16 limb product (q_hi+q_lo)(k_hi+k_lo) in 2 K=128
PE passes sharing one stationary tile: [q_hi;q_lo] against [k_hi;k_lo]
then [k_lo;k_hi] — constant row-size keeps the PE at its full ~0.42
ns/col streaming rate. x arrives pre-transposed (hi/lo fp16) from the
host, eliminating phase-A PE transposes.
Numerics vs fp32 CPU reference (numpy sim + HW): zero threshold flips,
rel err ~3.8e-3 (gate 2e-2).
"""
import os
import sys
import functools

import numpy as np

for _p in ("/opt/trn_rl_repo", "/root/.axon_site", "/root/.axon_site/_ro/trn_rl_repo"):
    if os.path.isdir(_p) and _p not in sys.path:
        sys.path.append(_p)

import ml_dtypes
from contextlib import ExitStack

import concourse.bass as bass
import concourse.bacc as bacc
import concourse.mybir as mybir
import concourse.tile as tile
from concourse import bass_utils

FP32 = mybir.dt.float32
FP16 = mybir.dt.float16
BF16 = mybir.dt.bfloat16
ALU = mybir.AluOpType
AFT = mybir.ActivationFunctionType


def _register_mask_scale_op():
    """One-pass masked normalize: out = in0 * s1 if in0 > s0 else 0.

    s0 = 0.01*Z and s1 = 1/Z are per-partition scalars (queries on
    partitions), so the compare is fp32-exact against the fp32 Z sum.
    """
    from concourse.dve_spec import Spec, Src0, C0, C1, Zero, select
    from concourse import dve_ops as dops

    name = "MASK_SCALE_GT_ANT"
    for op in dops.OPS:
        if op.name == name:
            return op
    op = dops.DveOp(
        name,
        Spec(
            body=select(C0 < Src0, Src0 * C1, Zero),
            reference=lambda in0, in1, s0, s1, imm2: np.where(
                s0 < in0, in0 * s1, 0.0).astype(np.float32),
        ),
        subdim=False,
        uops_sha={"v3": "5c524e60b0820e49", "v4": "43b563f88d645e85"},
    )
    dops.OPS.append(op)
    dops._SUB_OPCODE_FOR_NAME[name] = dops._CUSTOM_DVE_ROW_BASE + len(dops.OPS) - 1
    dops.CUSTOM_DVE_SPECS[name] = op.spec
    return op


MASK_OP = _register_mask_scale_op()


def emit_core_kernel(ctx, tc, io, n=2048, dim=512, hc=4, dh=64):
    nc = tc.nc
    inner = hc * dh                 # 256
    NT = n // 128                   # row tiles of n
    DC = dim // 128                 # contraction chunks of dim
    MH = inner // 128               # m-tiles of attnT (head pairs)
    MQK = 2 * MH                    # m-tiles of stacked [q;k]
    KB = n // 128                   # key blocks (128)
    KC = n // 512                   # key chunks (512)
    QG = n // 512                   # query groups of 512
    scale = dh ** -0.5

    # ---------------- constants ----------------
    cpool = ctx.enter_context(tc.tile_pool(name="consts", bufs=1))
    wqk_x = [None] * (2 * DC)
    wv_h = []
    for c in [*range(DC, 2 * DC), *range(DC)]:  # whi half first
        t = cpool.tile([128, 2 * inner], FP16, tag=f"wqkx{c}", name=f"wqkx{c}")
        nc.sync.dma_start(t[:], io["wqk_x"][c * 128:(c + 1) * 128, :])
        wqk_x[c] = t
    for c in range(DC):
        t = cpool.tile([128, inner], FP16, tag=f"wvh{c}", name=f"wvh{c}")
        nc.gpsimd.dma_start(t[:], io["wv_h"][c * 128:(c + 1) * 128, :])
        wv_h.append(t)
    wqk_h = wqk_x[DC:]  # whi rows of [wlo; whi]
    wout = []
    for m in range(MH):
        t = cpool.tile([128, dim], BF16, tag=f"wout{m}", name=f"wout{m}")
        nc.gpsimd.dma_start(t[:], io["wout_b"][m * 128:(m + 1) * 128, :])
        wout.append(t)
    bqk = []
    for m in range(MQK):
        t = cpool.tile([128, 1], FP32, tag=f"bqk{m}", name=f"bqk{m}")
        nc.gpsimd.dma_start(t[:], io["bqk"][m * 128:(m + 1) * 128, :])
        bqk.append(t)
    bv_bc = cpool.tile([128, inner], FP32, tag="bv", name="bv_bc")
    nc.gpsimd.dma_start(bv_bc[:], io["bv"][:])

    # persistent activations
    apool = ctx.enter_context(tc.tile_pool(name="acts", bufs=1))
    # per head: Q2=[q_hi;q_lo], KA=[k_hi;k_lo], KB=[k_lo;k_hi].
    # S = Q2^T KA + Q2^T KB = (q_hi+q_lo)(k_hi+k_lo): both passes K=128
    # with the SAME stationary tile — the PE pays its ~107ns row-size
    # reconfiguration penalty on K-alternating streams, so no K=64 pass.
    Q2 = [apool.tile([128, n], FP16, tag=f"Q2{h}", name=f"Q2{h}")
          for h in range(hc)]
    K_A = [apool.tile([128, n], FP16, tag=f"KA{h}", name=f"KA{h}")
           for h in range(hc)]
    K_B = [apool.tile([128, n], FP16, tag=f"KB{h}", name=f"KB{h}")
           for h in range(hc)]
    V_sb = [apool.tile([128, inner], BF16, tag=f"V{t}", name=f"V{t}")
            for t in range(NT)]
    attnT = [apool.tile([128, n], BF16, tag=f"attnT{m}", name=f"attnT{m}")
             for m in range(MH)]

    # ---------------- phase B: projections ----------------
    nqs = 512
    with tc.tile_pool(name="xT", bufs=1) as xtp:
        xTh = [xtp.tile([128, n], FP16, tag=f"xTh{c}", name=f"xTh{c}")
               for c in range(DC)]
        xTl = [xtp.tile([128, n], FP16, tag=f"xTl{c}", name=f"xTl{c}")
               for c in range(DC)]
        # x loads on the Act hwdge queue: the sync queue's serialized
        # ~680ns/dispatch would otherwise gate the first matmuls
        for c in range(DC):
            nc.scalar.dma_start(xTh[c][:], io["xTh"][c * 128:(c + 1) * 128, :])
        for c in range(DC):
            nc.scalar.dma_start(xTl[c][:], io["xTl"][c * 128:(c + 1) * 128, :])
        with tc.tile_pool(name="psB", bufs=4, space="PSUM") as psB, \
             tc.tile_pool(name="hilo", bufs=3) as hlp:
            MS = (0, MH, 1, MH + 1)  # q0,k0 first per chunk
            for nq in range(n // nqs):
                sl = slice(nq * nqs, (nq + 1) * nqs)
                pss = {}
                # all four m-chains' xTh-dependent matmuls first (8 of 12
                # each) across 4 psum banks, so the late xTl load is hidden
                for m in MS:
                    msl = slice(m * 128, (m + 1) * 128)
                    ps = pss[m] = psB.tile([128, nqs], FP32, tag="psB",
                                           name=f"psB{m}")
                    for c in range(DC):
                        nc.tensor.matmul(ps[:], wqk_h[c][:, msl],
                                         xTh[c][:, sl],
                                         start=(c == 0), stop=False)
                    for c2 in range(DC):
                        nc.tensor.matmul(ps[:], wqk_x[c2][:, msl],
                                         xTh[c2][:, sl],
                                         start=False, stop=False)
                for m in MS:
                    msl = slice(m * 128, (m + 1) * 128)
                    ps = pss[m]
                    for c2 in range(DC, 2 * DC):
                        nc.tensor.matmul(ps[:], wqk_x[c2][:, msl],
                                         xTl[c2 - DC][:, sl],
                                         start=False, stop=(c2 == 2 * DC - 1))
                    # aligned full-128 hi/lo extraction, then DMA moves
                    # (DVE cannot shift partition offsets; DMA can)
                    thi = hlp.tile([128, nqs], FP16, tag="thi")
                    tlo = hlp.tile([128, nqs], FP16, tag="tlo")
                    nc.vector.tensor_scalar(thi[:], ps[:], bqk[m][:], None,
                                            ALU.add)
                    nc.vector.scalar_tensor_tensor(tlo[:], ps[:], bqk[m][:],
                                                   thi[:], ALU.add,
                                                   ALU.subtract)
                    for hh in range(2):
                        h = 2 * (m % 2) + hh
                        rsl = slice(hh * 64, (hh + 1) * 64)
                        if m < MH:  # q: Q2=[q_hi;q_lo]
                            nc.sync.dma_start(Q2[h][0:64, sl], thi[rsl, :])
                            nc.gpsimd.dma_start(Q2[h][64:128, sl], tlo[rsl, :])
                        else:       # k: KA=[k_hi;k_lo], KB=[k_lo;k_hi]
                            nc.sync.dma_start(K_A[h][0:64, sl], thi[rsl, :])
                            nc.scalar.dma_start(K_B[h][64:128, sl], thi[rsl, :])
                            nc.gpsimd.dma_start(K_A[h][64:128, sl], tlo[rsl, :])
                            nc.gpsimd.dma_start(K_B[h][0:64, sl], tlo[rsl, :])
            # V natural [n, inner] bf16; bias added during psum drain
            for nt in range(NT):
                ps = psB.tile([128, inner], FP32, tag="psBv")
                tsl = slice(nt * 128, (nt + 1) * 128)
                for c in range(DC):
                    nc.tensor.matmul(ps[:], xTh[c][:, tsl], wv_h[c][:],
                                     start=(c == 0), stop=(c == DC - 1))
                nc.vector.tensor_tensor(V_sb[nt][:], ps[:], bv_bc[:], ALU.add)

    # ---------------- phase C: attention + output projection ----------
    with tc.tile_pool(name="psS", bufs=3, space="PSUM") as psS, \
         tc.tile_pool(name="psO", bufs=1, space="PSUM") as psOp, \
         tc.tile_pool(name="psE", bufs=1, space="PSUM") as psEp, \
         tc.tile_pool(name="Epool", bufs=6, space="SBUF") as Ep, \
         tc.tile_pool(name="Ppool", bufs=5, space="SBUF") as Pp, \
         tc.tile_pool(name="PTpool", bufs=3, space="SBUF") as PTp, \
         tc.tile_pool(name="zcr", bufs=4) as zp, \
         tc.tile_pool(name="ostage", bufs=4) as osp:

        last_item = [False]

        def stage_s(h, qt, accI):
            """S matmuls + exp(+Z accum) for one 128-query tile; returns
            its E tile. Z bookkeeping is batched per item in stage_m."""
            qsl = slice(qt * 128, (qt + 1) * 128)
            qt4 = qt % 4
            Et = Ep.tile([128, n], FP32, tag="E")
            for kk in range(n // 1024):
                ss = psS.tile([128, 1024], FP32, tag="S")
                for j in range(2):
                    kcsl = slice((2 * kk + j) * 512, (2 * kk + j + 1) * 512)
                    out = ss[:, j * 512:(j + 1) * 512]
                    nc.tensor.matmul(out, Q2[h][:, qsl], K_A[h][:, kcsl],
                                     start=True, stop=False)
                    nc.tensor.matmul(out, Q2[h][:, qsl], K_B[h][:, kcsl],
                                     start=False, stop=True)
                nc.scalar.activation(Et[:, kk * 1024:(kk + 1) * 1024], ss[:],
                                     AFT.Exp, scale=scale,
                                     accum_out=accI[:, 2 * qt4 + kk:
                                                    2 * qt4 + kk + 1])
            return Et

        def stage_m(accI, E4):
            """Batched z/c/r for the item's 4 query tiles, then the 4
            mask+scale ops and XBAR transposes."""
            z4 = zp.tile([128, 4], FP32, tag="z4")
            nc.vector.tensor_reduce(
                z4[:], accI[:].rearrange("p (q k) -> p q k", q=4),
                mybir.AxisListType.X, ALU.add)
            c4 = zp.tile([128, 4], FP32, tag="c4")
            nc.vector.tensor_scalar(c4[:], z4[:], 0.01, None, ALU.mult)
            r4 = zp.tile([128, 4], FP32, tag="r4")
            nc.vector.reciprocal_approx_fast(out=r4[:], in_=z4[:])
            pt3 = PT_cur[0][:].rearrange("p (kb q) -> p kb q", kb=KB)
            for qt4, Et in enumerate(E4):
                Pt = Pp.tile([128, n], BF16, tag="P")
                nc.vector._custom_dve(MASK_OP, out=Pt[:], in0=Et[:],
                                      s0=c4[:, qt4:qt4 + 1],
                                      s1=r4[:, qt4:qt4 + 1])
                eng = nc.scalar if (last_item[0] and qt4 % 2) else nc.sync
                eng.dma_start(pt3[:, :, qt4 * 128:(qt4 + 1) * 128], Pt[:],
                              transpose=True)

        def stage_pv(h, qg, PTt, split=False):
            """PV matmuls + attnT copy for one (head, 512-query group).
            split=True runs one accumulation chain per 128-query sub-tile
            (sharing the psO zero region) so the tail PV starts as soon as
            each transpose lands instead of after all four."""
            psO = psOp.tile([64, 512], FP32, tag="O")
            if split:
                for qt4 in range(4):
                    qsl = slice(qt4 * 128, (qt4 + 1) * 128)
                    for kc in range(KB):
                        nc.tensor.matmul(
                            psO[:, qsl],
                            V_sb[kc][:, h * dh:(h + 1) * dh],
                            PTt[:, kc * 512 + qt4 * 128:
                                kc * 512 + (qt4 + 1) * 128],
                            start=(kc == 0 and qt4 == 0), stop=(kc == KB - 1),
                            skip_group_check=True)
            else:
                for kc in range(KB):
                    nc.tensor.matmul(psO[:], V_sb[kc][:, h * dh:(h + 1) * dh],
                                     PTt[:, kc * 512:(kc + 1) * 512],
                                     start=(kc == 0), stop=(kc == KB - 1))
            mq, rq = h // 2, 64 * (h % 2)
            nc.vector.tensor_copy(
                attnT[mq][rq:rq + 64, qg * 512:(qg + 1) * 512], psO[:])

        def stage_out(qg, nt4, tail=False):
            """Output projection for one 128-row tile of query group qg."""
            nt = qg * 4 + nt4
            tsl = slice(nt * 128, (nt + 1) * 128)
            ps = psEp.tile([128, dim], FP32, tag="psE")
            for m in range(MH):
                nc.tensor.matmul(ps[:], attnT[m][:, tsl], wout[m][:],
                                 start=(m == 0), stop=(m == MH - 1))
            ot = osp.tile([128, dim], FP32, tag="ostage")
            nc.vector.tensor_copy(ot[:], ps[:])
            # tail pieces ride the by-then-idle sync hwdge queue
            eng = nc.sync if tail else nc.gpsimd
            eng.dma_start(io["out"][tsl, :], ot[:])

        items = [(qg, h) for qg in range(QG) for h in range(hc)]
        PT_cur = [None]
        prev = None  # (h, qg, PTt)
        for i, (qg, h) in enumerate(items):
            last_item[0] = (i == len(items) - 1)
            PT_cur[0] = PTp.tile([128, KB * 512], BF16, tag="PT", name="PT")
            accI = zp.tile([128, 8], FP32, tag="acc", name="accI")
            E4 = [stage_s(h, qg * 4 + qt4, accI) for qt4 in range(4)]
            stage_m(accI, E4)
            if prev is not None:
                stage_pv(*prev)
            if i >= 5:  # out-proj piece i-5: one extra item of lag so the
                j = i - 5  # needed attnT copy is already drained
                stage_out(j // 4, j % 4)
            prev = (h, qg, PT_cur[0])
        stage_pv(*prev, split=True)
        for j in range(len(items) - 5, len(items)):
            stage_out(j // 4, j % 4, tail=True)


def build_program(n=2048, dim=512, hc=4, dh=64):
    nc = bacc.Bacc(trn_type="TRN2", target_bir_lowering=False, debug=False)
    inner = hc * dh
    io = {}

    def din(name, shape, dt):
        io[name] = nc.dram_tensor(name, shape, dt, kind="ExternalInput").ap()

    din("xTh", [dim, n], FP16)
    din("xTl", [dim, n], FP16)
    din("wqk_x", [2 * dim, 2 * inner], FP16)
    din("wv_h", [dim, inner], FP16)
    din("bqk", [2 * inner, 1], FP32)
    din("bv", [128, inner], FP32)
    din("wout_b", [inner, dim], BF16)
    io["out"] = nc.dram_tensor("out", [n, dim], FP32, kind="ExternalOutput").ap()

    with tile.TileContext(nc) as tc:
        with ExitStack() as ctx:
            emit_core_kernel(ctx, tc, io, n=n, dim=dim, hc=hc, dh=dh)
    nc.compile()
    return nc


def make_core_inputs(x_b, Wq, Wk, Wv, bq, bk, bv, Wout_g, n=2048, dim=512,
                     hc=4, dh=64):
    f16 = np.float16
    inner = hc * dh
    xh = x_b.astype(f16)
    xl = (x_b - xh.astype(np.float32)).astype(f16)
    wqk = np.concatenate([Wq, Wk], axis=1)              # [dim, 2*inner]
    wqk_hi = wqk.astype(f16)
    wqk_lo = (wqk - wqk_hi.astype(np.float32)).astype(f16)
    wqk_x = np.concatenate([wqk_lo, wqk_hi], axis=0)    # [2*dim, 2*inner]
    return {
        "xTh": np.ascontiguousarray(xh.T),
        "xTl": np.ascontiguousarray(xl.T),
        "wqk_x": wqk_x,
        "wv_h": Wv.astype(f16),
        "bqk": np.concatenate([bq, bk]).reshape(2 * inner, 1).astype(np.float32),
        "bv": np.broadcast_to(bv.reshape(1, inner),
                              (128, inner)).astype(np.float32),
        "wout_b": Wout_g.astype(ml_dtypes.bfloat16),
    }


@functools.lru_cache(maxsize=1)
def _cached_program():
    return build_program()


def kernel(x, Wqkv, bqkv, Wout, bout):
    x = np.asarray(x, dtype=np.float32)
    Wqkv = np.asarray(Wqkv, dtype=np.float32)
    bqkv = np.asarray(bqkv, dtype=np.float32)
    Wout = np.asarray(Wout, dtype=np.float32)
    bout = np.asarray(bout, dtype=np.float32)

    b, n, dim = x.shape
    H, dh = 8, 64
    inner = H * dh
    hc = 4
    Wq, Wk, Wv = Wqkv[:, :inner], Wqkv[:, inner:2 * inner], Wqkv[:, 2 * inner:]
    bq, bk, bv = bqkv[:inner], bqkv[inner:2 * inner], bqkv[2 * inner:]

    in_maps = []
    for c in range(8):
        bb, g = c // 2, c % 2
        hsl = slice(g * hc * dh, (g + 1) * hc * dh)
        in_maps.append(make_core_inputs(
            x[bb], Wq[:, hsl], Wk[:, hsl], Wv[:, hsl],
            bq[hsl], bk[hsl], bv[hsl], Wout[hsl, :],
            n=n, dim=dim, hc=hc, dh=dh))

    nc = _cached_program()
    res = bass_utils.run_bass_kernel_spmd(nc, in_maps, core_ids=list(range(8)))
    global LAST_RESULTS
    LAST_RESULTS = res
    out = np.empty((b, n, dim), dtype=np.float32)
    for bb in range(b):
        out[bb] = res.results[2 * bb]["out"] + res.results[2 * bb + 1]["out"] \
            + bout
    return out
